# revision 1
# baseline (speedup 1.0000x reference)
"""CP-decomposed conv (pointwise -> depthwise-h -> depthwise-w -> pointwise)
as a Bass/Tile kernel on 8 TRN2 NeuronCores.

Strategy:
  - Data-parallel over batch: 32 images -> 4 per core, no collectives.
  - fp16 wire format: x and out cross HBM as fp16 (halves DMA bytes, the
    roofline); accumulation is always fp32 in PSUM. Host casts back.
  - Fold the depthwise h-conv into the first pointwise conv:
      y2[r,i,w] = sum_{h,c} (factor3[c,r]*factor1[h,r]) * x[c,i+h,w]
    -> 6 accumulating fp16 matmuls per PSUM tile (3 h-shifts x 2 C-chunks).
    y2 stays in PSUM.
  - Depthwise w-conv straight out of PSUM on ACT+DVE with per-partition
    scalars (factor2[w,r] lives on partition r):
      y3 = sum_w y2[:,:,w:w+94] * f2[w]   (1 ACT copy-scale + 2 DVE STT)
  - Final projection R->F in float32r: one matmul per (fc, row-tile).
  - Input DMAs via SWDGE on the idle GpSimd engine, outputs on SP HWDGE.
"""

import sys
import numpy as np

for _p in ("/opt/trn_rl_repo",):
    if _p not in sys.path:
        sys.path.insert(0, _p)

B, C, H, W = 32, 256, 96, 96
F, FH, FW, R = 512, 3, 3, 128
OH, OW = H - FH + 1, W - FW + 1  # 94, 94
NCORES = 8
BLOC = B // NCORES  # 4 images per core


# output-row strips per image and row-tiles within a strip (all row-tiles
# >= 3 rows so every matmul free dim >= 256 -> full float32r rate).
def _strips(s_list):
    out, i0 = [], 0
    for s in s_list:
        out.append((i0, s))
        i0 += s
    assert i0 == OH
    return out


STRIPS_BY_IMG = {
    0: _strips([12, 12, 23, 47]),
    1: _strips([47, 47]),
    2: _strips([47, 47]),
    3: _strips([47, 23, 12, 12]),
}
ROW_TILES_BY_S = {
    47: [5] * 8 + [4, 3],
    23: [5, 5, 5, 5, 3],
    12: [4, 4, 4],
}


def _col_tiles(total):
    """Tile a flat column count into chunks of <=512, all >=256."""
    out = []
    left = total
    while left > 0:
        if left >= 512 + 256 or left <= 512:
            t = min(512, left)
        else:
            t = left - 256
        out.append(t)
        left -= t
    assert sum(out) == total and all(256 <= t <= 512 for t in out[:-1])
    return out

_NC_CACHE = {}


def _build_nc():
    import concourse.bacc as bacc
    import concourse.mybir as mybir
    import concourse.tile as tile

    f32 = mybir.dt.float32
    f32r = mybir.dt.float32r
    f16 = mybir.dt.float16
    mult = mybir.AluOpType.mult
    add = mybir.AluOpType.add

    nc = bacc.Bacc("TRN2", target_bir_lowering=False, debug=True)

    xd = nc.dram_tensor("x", [BLOC, C, H, W], f16, kind="ExternalInput")
    # wab packs the 6 stage-A weight tiles (h, chunk) then the 4 stage-D
    # tiles (fc): [10, 128, 128] fp16, loaded in ONE dma
    wabd = nc.dram_tensor("wab", [10, 128, 128], f16, kind="ExternalInput")
    wcd = nc.dram_tensor("wc", [R, FW], f32, kind="ExternalInput")
    od = nc.dram_tensor("out", [BLOC, F, OH, OW], f16, kind="ExternalOutput")

    with tile.TileContext(nc) as tc:
        with (
            tc.tile_pool(name="wpool", bufs=1) as wpool,
            tc.tile_pool(name="xs", bufs=4) as xs_pool,
            tc.tile_pool(name="y3", bufs=3) as y3_pool,
            tc.tile_pool(name="osb", bufs=5) as osb_pool,
            tc.tile_pool(name="psA", bufs=4, space="PSUM") as psA,
            tc.tile_pool(name="psD", bufs=4, space="PSUM") as psD,
        ):
            # wc first: the very first stage-C op depends on it
            wc_sb = wpool.tile([128, FW], f32)
            nc.sync.dma_start(wc_sb[:], wcd[:])
            wab_sb = wpool.tile([128, 10, 128], f16)
            nc.sync.dma_start(
                wab_sb[:], wabd.ap().rearrange("t p c -> p t c")
            )
            wa_sb = wab_sb  # [:, h*2+ch, :] for stage A
            wb_off = FH * 2  # wab_sb[:, wb_off+fc, :] for stage D

            copy_i = 0  # alternate stage-D PSUM->SBUF copies DVE/ACT

            def psum_copy(dst, src):
                nonlocal copy_i
                if copy_i % 3 == 0:
                    nc.vector.tensor_copy(dst, src)
                else:
                    nc.scalar.copy(dst, src)
                copy_i += 1

            for b in range(BLOC):
                for i0, S in STRIPS_BY_IMG[b]:
                    nrows_in = S + 2  # x halo
                    xs_t = xs_pool.tile([128, 2, nrows_in * W], f16)
                    for ch in range(2):
                        # input loads via SWDGE on the idle GpSimd engine;
                        # output stores on the SP HWDGE ring
                        nc.gpsimd.dma_start(
                            xs_t[:, ch, :],
                            xd[b, ch * 128 : (ch + 1) * 128, i0 : i0 + nrows_in, :],
                        )

                    y3_t = y3_pool.tile([128, S * OW], f16)

                    # stage A+B: pointwise C->R with h-conv folded in (PSUM),
                    # then stage C: w-conv PSUM->SBUF via per-partition scalars
                    row_tiles = ROW_TILES_BY_S[S]
                    r0 = 0
                    for nr in row_tiles:
                        pa = psA.tile([128, nr, W], f32)
                        k = 0
                        for h in range(FH):
                            for ch in range(2):
                                nc.tensor.matmul(
                                    pa[:],
                                    wa_sb[:, h * 2 + ch, :],
                                    xs_t[:, ch, (r0 + h) * W : (r0 + h + nr) * W],
                                    start=(k == 0),
                                    stop=(k == 5),
                                )
                                k += 1
                        dst = y3_t[:, r0 * OW : (r0 + nr) * OW]
                        nc.scalar.mul(dst, pa[:, :, 0:OW], wc_sb[:, 0:1])
                        nc.vector.scalar_tensor_tensor(
                            dst, pa[:, :, 1 : 1 + OW], wc_sb[:, 1:2], dst,
                            op0=mult, op1=add,
                        )
                        nc.vector.scalar_tensor_tensor(
                            dst, pa[:, :, 2 : 2 + OW], wc_sb[:, 2:3], dst,
                            op0=mult, op1=add,
                        )
                        r0 += nr

                    # stage D: projection R->F over flat 512-col tiles of y3
                    col_tiles = _col_tiles(S * OW)
                    for fc in range(4):
                        ot = osb_pool.tile([128, S * OW], f16)
                        c0 = 0
                        for nt in col_tiles:
                            pd = psD.tile([128, 512], f32)
                            nc.tensor.matmul(
                                pd[:, 0:nt],
                                wab_sb[:, wb_off + fc, :],
                                y3_t[:, c0 : c0 + nt],
                                start=True,
                                stop=True,
                            )
                            psum_copy(ot[:, c0 : c0 + nt], pd[:, 0:nt])
                            c0 += nt
                        nc.sync.dma_start(
                            od[b, fc * 128 : (fc + 1) * 128, i0 : i0 + S, :],
                            ot[:],
                        )

    nc.compile()
    return nc


def _get_nc():
    if "nc" not in _NC_CACHE:
        _NC_CACHE["nc"] = _build_nc()
    return _NC_CACHE["nc"]


def _prep_weights(factor0, factor1, factor2, factor3):
    # wab[0:6] = stage-A tiles: [h*2+ch, c', r] = factor3[ch*128+c', r]*factor1[h, r]
    # wab[6:10] = stage-D tiles: [fc, r, f'] = factor0[fc*128+f', r]
    wa = (factor3[None, :, :] * factor1[:, None, :]).reshape(FH, 2, 128, R)
    wb = factor0.reshape(4, 128, R).transpose(0, 2, 1)
    wab = np.concatenate(
        [wa.reshape(6, 128, R), wb], axis=0
    ).astype(np.float16)
    wab = np.ascontiguousarray(wab)
    # wc[r, w] = factor2[w, r]
    wc = np.ascontiguousarray(factor2.T, dtype=np.float32)
    return wab, wc


def _prep_x(x):
    return np.ascontiguousarray(x).astype(np.float16)


def kernel(x, factor0, factor1, factor2, factor3):
    from concourse import bass_utils

    x = np.asarray(x, dtype=np.float32)
    factor0 = np.asarray(factor0, dtype=np.float32)
    factor1 = np.asarray(factor1, dtype=np.float32)
    factor2 = np.asarray(factor2, dtype=np.float32)
    factor3 = np.asarray(factor3, dtype=np.float32)

    wab, wc = _prep_weights(factor0, factor1, factor2, factor3)
    x16 = _prep_x(x)

    nc = _get_nc()
    in_maps = [
        {"x": x16[c * BLOC : (c + 1) * BLOC], "wab": wab, "wc": wc}
        for c in range(NCORES)
    ]
    res = bass_utils.run_bass_kernel_spmd(nc, in_maps, list(range(NCORES)))
    out = np.concatenate(
        [res.results[c]["out"] for c in range(NCORES)], axis=0
    )
    return out.astype(np.float32)

